# revision 45
# baseline (speedup 1.0000x reference)
"""Trainium2 Bass kernel for nn_CrossAttentionLayer (m=n=1024, d=2048), fp8.

Math: f = relu(term1 + term23 + term4 + ffn_b), with W1..W4 = ffn_w.reshape(n,4,d):
  term1  = sum_i u_p[i] . W1[i]                              (host, f64)
  term23 = sum_i [sum_k E[i,k] M2[i,k]] / [sum_k E[i,k]]     (row softmax)
  term4  = sum_k [sum_i E[i,k] M4[i,k]] / [sum_i E[i,k]]     (col softmax)
where E = exp(S - 6),  S[i,k] = u_p[i].w1 + u_c[k].w2 + (u_p[i]*w3).u_c[k],
  M2 = (W2 + u_p*W3) @ u_c.T = C @ u_c.T,   M4 = u_p @ (u_c*W4).T = u_p @ V4.T.
Softmax ratios cancel per-row/col/global shifts, so ONE exp(S) array serves
both softmaxes; the global -6 keeps exp within fp16 range.

All matmul operands are fp8e4 (TRN E4M3, max 240) with host-side scaling to
land in the format's sweet spot: uct=u_c, upt=u_p (unit scale), cmt=512*C,
v4t=512*V4, and on-device asp = 16*(u_p*w3 + w2) (w2/w3 folded per-d-chunk
scalars via tensor_scalar as upt chunks land; the exp activation divides
the logits back via scale=1/16 and adds the row bias r_i = u_p[i].w1 - 6
per partition).  Matmuls run in DoubleRow perf mode: each instruction
consumes TWO 128-deep contraction chunks at 2x fp16 throughput.  PSUM
accumulates in f32; the final scalar has ~0.008 absolute error against f64
vs a -1.37 pre-relu margin (the output relu-clamps to 0 exactly).

Sharding: 8 cores = 4 mention shards (I, 256 rows) x 2 candidate shards
(K, 512 cols); each core emits its [256,512] blocks of S/M2/M4, reduces
  Z[i] = sum_k E,  G[i] = sum_k E*(512*M2)   (activation/STT accumulators)
  Z'[k] = sum_i E, N[k] = sum_i E*(512*M4)   (ones-vector fp16 matmuls)
and the host sums the per-core partials in f64 and applies bias + relu.

Schedule: the two HWDGE queues carry ~1.5MB each (wvr, with the asp
scalars and exp bias, FIRST on sync -- the gpsimd SWDGE queue's first
byte lands ~4us later and wvr gates the asp derivation); the S phase
(tps) runs first so exp fires mid-stream, then M4 (qps, long h4->ncp->nz
tail chain) and M2 (mps, short h2->zg tail) track the v4t/cmt arrivals.
Warm-up matmuls ramp the PE activity window before real data lands.
"""

import sys

sys.path.insert(0, "/opt/trn_rl_repo")

import numpy as np
import ml_dtypes

import concourse.bass as bass
import concourse.tile as tile
from concourse import mybir
from concourse.bass_utils import run_bass_kernel_spmd

F32 = mybir.dt.float32
F16 = mybir.dt.float16
F8 = mybir.dt.float8e4
NPF8 = ml_dtypes.float8_e4m3
DR = mybir.MatmulPerfMode.DoubleRow

M = 1024  # mentions
N = 1024  # candidates
D = 2048  # feature dim (contraction)
NCORES = 8
ISH = 256  # mention rows per core
KSH = 512  # candidate cols per core
NI = M // ISH  # 4 mention shards
NK = N // KSH  # 2 candidate shards
CH = D // 128  # 16 contraction chunks
ITILES = ISH // 128  # 2
SC_CV = 512.0  # host scale on C and V4 (elements ~1e-3 -> fp8 sweet spot)
SC_A = 16.0  # device scale on asp = u_p*w3 + w2 (elements ~3e-2)

# ---------------------------------------------------------------------------
# Workaround: the pinned neuronxcc walrus accepts fewer sync waits per
# instruction than Tile's semaphore assignment attaches.  After scheduling,
# hoist excess waits of any over-capacity instruction onto same-engine
# EventSemaphores inserted right before it; each engine executes its stream
# in order, so the waits still gate the instruction.
_DEFAULT_CAP = 1
_wfix_counter = [0]


def _legalize_waits(nc: bass.Bass) -> None:
    for f in nc.m.functions:
        for bb in f.blocks:
            il = bb.instructions
            out = []
            for inst in il:
                si = inst.sync_info
                waits = list(si.on_wait) if si and si.on_wait else []
                if len(waits) > _DEFAULT_CAP:
                    keep = waits[:_DEFAULT_CAP]
                    for w in waits[_DEFAULT_CAP:]:
                        _wfix_counter[0] += 1
                        out.append(
                            mybir.InstEventSemaphore(
                                name=f"I-wfix-{_wfix_counter[0]}",
                                engine=inst.engine,
                                ins=[],
                                outs=[],
                                sync_info=mybir.SyncInfo(on_wait=[w], on_update=[]),
                            )
                        )
                    inst.sync_info = mybir.SyncInfo(
                        on_wait=keep, on_update=list(si.on_update or [])
                    )
                out.append(inst)
            bb.instructions = out


# ---------------------------------------------------------------------------
def _emit(nc: bass.Bass, tc: tile.TileContext, io: dict) -> None:
    mult = mybir.AluOpType.mult
    add = mybir.AluOpType.add

    upt_r = io["upt"].ap().rearrange("p (c i) -> p c i", c=CH)
    cmt_r = io["cmt"].ap().rearrange("p (c i) -> p c i", c=CH)
    uct_r = io["uct"].ap().rearrange("p (c k) -> p c k", c=CH)
    v4t_r = io["v4t"].ap().rearrange("p (c k) -> p c k", c=CH)
    wvr_r = io["wvr"].ap()

    import contextlib

    ctx = contextlib.ExitStack()
    singles = ctx.enter_context(tc.tile_pool(name="singles", bufs=1))
    # bufs=4 so the h4/h2 product tiles get distinct buffers -- with 2, the
    # h2 writes pick up WAR edges against the ncp matmuls' h4 reads, which
    # chains the zg path behind the nz path at the very end of the kernel.
    scratch = ctx.enter_context(tc.tile_pool(name="scratch", bufs=4))
    psum = ctx.enter_context(tc.tile_pool(name="psum", bufs=1, space="PSUM"))

    upt = singles.tile([128, CH, ISH], F8)
    asp = singles.tile([128, CH, ISH], F8)
    cmt = singles.tile([128, CH, ISH], F8)
    uct = singles.tile([128, CH, KSH], F8)
    v4t = singles.tile([128, CH, KSH], F8)
    wvr = singles.tile([128, CH * 2 + ITILES], F32)
    wv = wvr[:, : CH * 2].rearrange("p (c v) -> p c v", c=CH)
    rb = wvr[:, CH * 2 :]
    ones = singles.tile([128, 1], F16)

    ev = [singles.tile([128, KSH], F16, name=f"ev{it}", tag=f"ev{it}") for it in range(ITILES)]
    zg = singles.tile([128, 2 * ITILES], F32)  # cols: Z it0, Z it1, G it0, G it1
    nzs = singles.tile([1, 2 * KSH], F32)

    # PSUM: 6 full-kernel accumulators + 2 column-sum banks = all 8 banks.
    tps = [psum.tile([128, KSH], F32, name=f"tps{it}", tag=f"tps{it}") for it in range(ITILES)]
    mps = [psum.tile([128, KSH], F32, name=f"mps{it}", tag=f"mps{it}") for it in range(ITILES)]
    qps = [psum.tile([128, KSH], F32, name=f"qps{it}", tag=f"qps{it}") for it in range(ITILES)]
    zcp = psum.tile([1, KSH], F32)
    ncp = psum.tile([1, KSH], F32)

    # Warm-up operands first so the dummy matmuls can start right after the
    # prologue and have the HAM activity window at full clock by the time
    # real data lands.
    dum = singles.tile([128, KSH], F16)
    nc.gpsimd.memset(ones, 1.0)
    nc.gpsimd.memset(dum, 0.0)
    for _ in range(8):
        nc.tensor.matmul(zcp, lhsT=ones, rhs=dum, start=True, stop=True)

    # Input DMAs.  wvr leads on the sync HWDGE queue (it gates the asp
    # derivation; the gpsimd SWDGE queue's first byte lands ~4us later);
    # the bulk fp8 operands split ~1.5MB/1.5MB across the two HWDGE queues
    # in consumption order: upt/uct (S path) first, v4t next, cmt last.
    nc.sync.dma_start(out=wvr, in_=wvr_r)
    nc.scalar.dma_start(out=upt[:, 0:8, :], in_=upt_r[:, 0:8, :])
    for c0, c1 in ((0, 4), (4, 8), (8, 12), (12, 16)):
        nc.sync.dma_start(out=uct[:, c0:c1, :], in_=uct_r[:, c0:c1, :])
    nc.scalar.dma_start(out=v4t[:, 0:4, :], in_=v4t_r[:, 0:4, :])
    nc.scalar.dma_start(out=upt[:, 8:16, :], in_=upt_r[:, 8:16, :])
    nc.scalar.dma_start(out=v4t[:, 4:8, :], in_=v4t_r[:, 4:8, :])
    nc.scalar.dma_start(out=v4t[:, 8:12, :], in_=v4t_r[:, 8:12, :])
    nc.scalar.dma_start(out=v4t[:, 12:16, :], in_=v4t_r[:, 12:16, :])
    nc.sync.dma_start(out=cmt[:, 0:8, :], in_=cmt_r[:, 0:8, :])
    nc.sync.dma_start(out=cmt[:, 8:16, :], in_=cmt_r[:, 8:16, :])

    # asp = 16*(u_p*w3 + w2) per-partition scalars, chunk by chunk as upt
    # lands (DVE computes in f32 internally, converts fp8 in/out).
    for c in range(CH):
        nc.vector.tensor_scalar(
            out=asp[:, c, :],
            in0=upt[:, c, :],
            scalar1=wv[:, c, 0:1],
            scalar2=wv[:, c, 1:2],
            op0=mult,
            op1=add,
        )

    # S (tps) and M4 (qps) contractions interleaved per chunk pair: the T
    # matmuls pace on the serial asp chain, and the Q matmuls (which only
    # need upt/v4t, both early in the queues) fill the PE's asp-wait gaps.
    for c in range(0, CH, 2):
        st = c == 0
        sp = c == CH - 2
        for it in range(ITILES):
            lhs = asp[:, c : c + 2, it * 128 : (it + 1) * 128]
            nc.tensor.matmul(
                tps[it], lhsT=lhs, rhs=uct[:, c : c + 2, :],
                start=st, stop=sp, perf_mode=DR,
            )
        for it in range(ITILES):
            lhs = upt[:, c : c + 2, it * 128 : (it + 1) * 128]
            nc.tensor.matmul(
                qps[it], lhsT=lhs, rhs=v4t[:, c : c + 2, :],
                start=st, stop=sp, perf_mode=DR,
            )

    # E = exp(T/16 + r) as soon as tps closes; row sums via activation accum.
    for it in range(ITILES):
        nc.scalar.activation(
            out=ev[it],
            in_=tps[it],
            func=mybir.ActivationFunctionType.Exp,
            bias=rb[:, it : it + 1],
            scale=1.0 / SC_A,
            accum_out=zg[:, it : it + 1],
        )

    # Z' = colsum(E) and h4 = E * (512*M4) follow the exps; the M2 phase
    # below overlaps them on the PE.
    for jt in range(ITILES):
        nc.tensor.matmul(
            zcp, lhsT=ones, rhs=ev[jt],
            start=(jt == 0), stop=(jt == ITILES - 1),
        )
    nc.scalar.activation(
        out=nzs[:, 0:KSH], in_=zcp, func=mybir.ActivationFunctionType.Copy
    )
    h4s = []
    for it in range(ITILES):
        h4 = scratch.tile([128, KSH], F16, tag=f"h4_{it}")
        nc.vector.scalar_tensor_tensor(
            out=h4, in0=ev[it], scalar=1.0, in1=qps[it], op0=mult, op1=mult,
        )
        h4s.append(h4)

    # M2 contraction (mps), row-tile-separated so h2[0] overlaps M_it1;
    # N' = colsum(h4), its copy and the nz DMA close the PE/scalar streams
    # while only h2[1] + the zg DMA follow the final DR matmul.
    for it in range(ITILES):
        for c in range(0, CH, 2):
            lhs = cmt[:, c : c + 2, it * 128 : (it + 1) * 128]
            nc.tensor.matmul(
                mps[it], lhsT=lhs, rhs=uct[:, c : c + 2, :],
                start=(c == 0), stop=(c == CH - 2), perf_mode=DR,
            )
        h2 = scratch.tile([128, KSH], F16, tag=f"h2_{it}")
        nc.vector.scalar_tensor_tensor(
            out=h2,
            in0=ev[it],
            scalar=1.0,
            in1=mps[it],
            op0=mult,
            op1=mult,
            accum_out=zg[:, ITILES + it : ITILES + it + 1],
        )
    nc.sync.dma_start(out=io["out_zg"].ap(), in_=zg)
    for jt in range(ITILES):
        nc.tensor.matmul(
            ncp, lhsT=ones, rhs=h4s[jt],
            start=(jt == 0), stop=(jt == ITILES - 1),
        )
    nc.scalar.activation(
        out=nzs[:, KSH:], in_=ncp, func=mybir.ActivationFunctionType.Copy
    )
    nc.scalar.dma_start(out=io["out_nz"].ap(), in_=nzs)
    ctx.close()


def _build() -> bass.Bass:
    nc = bass.Bass()
    io = {}
    io["upt"] = nc.declare_dram_parameter("upt", [128, CH * ISH], F8, isOutput=False)
    io["cmt"] = nc.declare_dram_parameter("cmt", [128, CH * ISH], F8, isOutput=False)
    io["uct"] = nc.declare_dram_parameter("uct", [128, CH * KSH], F8, isOutput=False)
    io["v4t"] = nc.declare_dram_parameter("v4t", [128, CH * KSH], F8, isOutput=False)
    io["wvr"] = nc.declare_dram_parameter(
        "wvr", [128, CH * 2 + ITILES], F32, isOutput=False
    )
    io["out_zg"] = nc.declare_dram_parameter(
        "out_zg", [128, 2 * ITILES], F32, isOutput=True
    )
    io["out_nz"] = nc.declare_dram_parameter(
        "out_nz", [1, 2 * KSH], F32, isOutput=True
    )
    with tile.TileContext(nc) as tc:
        _emit(nc, tc, io)
    _legalize_waits(nc)
    return nc


_NC_CACHE: bass.Bass | None = None


def _get_nc() -> bass.Bass:
    global _NC_CACHE
    if _NC_CACHE is None:
        _NC_CACHE = _build()
    return _NC_CACHE


def _pack8(a2d: np.ndarray) -> np.ndarray:
    """[D, x] (d-major) f32 -> [128, ch, x] p-major fp8 so each partition's
    data is one contiguous DRAM run per chunk group."""
    d, x = a2d.shape
    ch = d // 128
    v = np.ascontiguousarray(a2d.reshape(ch, 128, x).transpose(1, 0, 2))
    return np.clip(v, -240.0, 240.0).astype(NPF8)


def _in_maps(u_p, u_c, w_a, ffn_w):
    W = ffn_w.reshape(N, 4, D)
    wa = w_a[0]
    w1, w2, w3 = wa[:D], wa[D : 2 * D], wa[2 * D :]

    C = SC_CV * (W[:, 1, :] + u_p * W[:, 2, :])  # [m, d] scaled
    V4 = SC_CV * (u_c * W[:, 3, :])  # [n, d] scaled
    r = u_p @ w1 - 6.0  # [m] row bias, shifted into fp16-exp range

    u_pT = _pack8(u_p.T)
    u_cT = _pack8(u_c.T)
    CT = _pack8(C.T)
    V4T = _pack8(V4.T)
    wv = np.ascontiguousarray(
        np.stack([SC_A * w3, SC_A * w2], axis=1)
        .reshape(CH, 128, 2)
        .transpose(1, 0, 2)
        .astype(np.float32)
    )

    maps = []
    for ii in range(NI):
        isl = slice(ISH * ii, ISH * (ii + 1))
        rbl = np.ascontiguousarray(
            r[isl].astype(np.float32).reshape(ITILES, 128).T
        )
        wvr = np.ascontiguousarray(
            np.concatenate([wv.reshape(128, CH * 2), rbl], axis=1)
        )
        upt = np.ascontiguousarray(u_pT[:, :, isl]).reshape(128, -1)
        cmt = np.ascontiguousarray(CT[:, :, isl]).reshape(128, -1)
        for kk in range(NK):
            ksl = slice(KSH * kk, KSH * (kk + 1))
            maps.append(
                {
                    "upt": upt,
                    "cmt": cmt,
                    "uct": np.ascontiguousarray(u_cT[:, :, ksl]).reshape(128, -1),
                    "v4t": np.ascontiguousarray(V4T[:, :, ksl]).reshape(128, -1),
                    "wvr": wvr,
                }
            )
    return maps


def _reduce(results: list[dict], term1: float, ffn_b) -> float:
    """Pre-relu scalar from the per-core partial sums, in float64."""
    total = term1
    # term23: per I shard, Z/G summed over the 2 K cores, then sum_i G/Z.
    for ii in range(NI):
        zg0 = results[ii * NK]["out_zg"].astype(np.float64)
        zg1 = results[ii * NK + 1]["out_zg"].astype(np.float64)
        z = zg0[:, :ITILES] + zg1[:, :ITILES]
        g = (zg0[:, ITILES:] + zg1[:, ITILES:]) / SC_CV
        total += (g / z).sum()
    # term4: per K shard, Z'/N summed over the 4 I cores, then sum_k N/Z'.
    for kk in range(NK):
        acc = np.zeros((2 * KSH,), np.float64)
        for ii in range(NI):
            acc += results[ii * NK + kk]["out_nz"][0].astype(np.float64)
        total += (acc[KSH:] / SC_CV / acc[:KSH]).sum()
    return total + float(np.asarray(ffn_b)[0])


def kernel(u_p, u_c, w_a, ffn_w, ffn_b, **run_kwargs):
    nc = _get_nc()
    u_p = np.asarray(u_p, np.float64)
    u_c = np.asarray(u_c, np.float64)
    w_a = np.asarray(w_a, np.float64)
    ffn_w = np.asarray(ffn_w, np.float64)
    maps = _in_maps(u_p, u_c, w_a, ffn_w)
    term1 = float(np.sum(u_p * ffn_w.reshape(N, 4, D)[:, 0, :]))
    res = run_bass_kernel_spmd(nc, maps, core_ids=list(range(NCORES)), **run_kwargs)
    out = np.array([max(_reduce(res.results, term1, ffn_b), 0.0)], dtype=np.float32)
    if run_kwargs:
        return out, res
    return out


# revision 47
# speedup vs baseline: 1.0234x; 1.0234x over previous
"""Trainium2 Bass kernel for nn_CrossAttentionLayer (m=n=1024, d=2048), fp8.

Math: f = relu(term1 + term23 + term4 + ffn_b), with W1..W4 = ffn_w.reshape(n,4,d):
  term1  = sum_i u_p[i] . W1[i]                              (host, f64)
  term23 = sum_i [sum_k E[i,k] M2[i,k]] / [sum_k E[i,k]]     (row softmax)
  term4  = sum_k [sum_i E[i,k] M4[i,k]] / [sum_i E[i,k]]     (col softmax)
where E = exp(S - 6),  S[i,k] = u_p[i].w1 + u_c[k].w2 + (u_p[i]*w3).u_c[k],
  M2 = (W2 + u_p*W3) @ u_c.T = C @ u_c.T,   M4 = u_p @ (u_c*W4).T = u_p @ V4.T.
Softmax ratios cancel per-row/col/global shifts, so ONE exp(S) array serves
both softmaxes; the global -6 keeps exp within fp16 range.

All matmul operands are fp8e4 (TRN E4M3, max 240) with host-side scaling to
land in the format's sweet spot: uct=u_c, upt=u_p (unit scale), cmt=512*C,
v4t=512*V4, and on-device asp = 16*(u_p*w3 + w2) (w2/w3 folded per-d-chunk
scalars via tensor_scalar as upt chunks land; the exp activation divides
the logits back via scale=1/16 and adds the row bias r_i = u_p[i].w1 - 6
per partition).  Matmuls run in DoubleRow perf mode: each instruction
consumes TWO 128-deep contraction chunks at 2x fp16 throughput.  PSUM
accumulates in f32; the final scalar has ~0.008 absolute error against f64
vs a -1.37 pre-relu margin (the output relu-clamps to 0 exactly).

Sharding: 8 cores = 4 mention shards (I, 256 rows) x 2 candidate shards
(K, 512 cols); each core emits its [256,512] blocks of S/M2/M4, reduces
  Z[i] = sum_k E,  G[i] = sum_k E*(512*M2)   (activation/STT accumulators)
  Z'[k] = sum_i E, N[k] = sum_i E*(512*M4)   (ones-vector fp16 matmuls)
and the host sums the per-core partials in f64 and applies bias + relu.

Schedule: the two HWDGE queues carry ~1.5MB each (wvr, with the asp
scalars and exp bias, FIRST on sync -- the gpsimd SWDGE queue's first
byte lands ~4us later and wvr gates the asp derivation).  The S (tps)
and M4 (qps) contractions interleave per chunk pair on the in-order PE:
T paces on the serial asp chain, and the independent Q matmuls (upt and
v4t ride early queue slots) fill the PE's asp-wait gaps.  exp fires
mid-stream at the joint stop; M2 (mps, row-tile-separated) tracks the
cmt queue tails so the h4->ncp->nz chain overlaps it and only h2[1] +
the 2KB zg DMA follow the final matmul.  A short warm-up burst (8
matmuls -- more blocks the ready stream, fewer starts it cold) ramps
the PE activity window while the first transfers are in flight.
"""

import sys

sys.path.insert(0, "/opt/trn_rl_repo")

import numpy as np
import ml_dtypes

import concourse.bass as bass
import concourse.tile as tile
from concourse import mybir
from concourse.bass_utils import run_bass_kernel_spmd

F32 = mybir.dt.float32
F16 = mybir.dt.float16
F8 = mybir.dt.float8e4
NPF8 = ml_dtypes.float8_e4m3
DR = mybir.MatmulPerfMode.DoubleRow

M = 1024  # mentions
N = 1024  # candidates
D = 2048  # feature dim (contraction)
NCORES = 8
ISH = 256  # mention rows per core
KSH = 512  # candidate cols per core
NI = M // ISH  # 4 mention shards
NK = N // KSH  # 2 candidate shards
CH = D // 128  # 16 contraction chunks
ITILES = ISH // 128  # 2
SC_CV = 512.0  # host scale on C and V4 (elements ~1e-3 -> fp8 sweet spot)
SC_A = 16.0  # device scale on asp = u_p*w3 + w2 (elements ~3e-2)

# ---------------------------------------------------------------------------
# Workaround: the pinned neuronxcc walrus accepts fewer sync waits per
# instruction than Tile's semaphore assignment attaches.  After scheduling,
# hoist excess waits of any over-capacity instruction onto same-engine
# EventSemaphores inserted right before it; each engine executes its stream
# in order, so the waits still gate the instruction.
_DEFAULT_CAP = 1
_wfix_counter = [0]


def _legalize_waits(nc: bass.Bass) -> None:
    for f in nc.m.functions:
        for bb in f.blocks:
            il = bb.instructions
            out = []
            for inst in il:
                si = inst.sync_info
                waits = list(si.on_wait) if si and si.on_wait else []
                if len(waits) > _DEFAULT_CAP:
                    keep = waits[:_DEFAULT_CAP]
                    for w in waits[_DEFAULT_CAP:]:
                        _wfix_counter[0] += 1
                        out.append(
                            mybir.InstEventSemaphore(
                                name=f"I-wfix-{_wfix_counter[0]}",
                                engine=inst.engine,
                                ins=[],
                                outs=[],
                                sync_info=mybir.SyncInfo(on_wait=[w], on_update=[]),
                            )
                        )
                    inst.sync_info = mybir.SyncInfo(
                        on_wait=keep, on_update=list(si.on_update or [])
                    )
                out.append(inst)
            bb.instructions = out


# ---------------------------------------------------------------------------
def _emit(nc: bass.Bass, tc: tile.TileContext, io: dict) -> None:
    mult = mybir.AluOpType.mult
    add = mybir.AluOpType.add

    upt_r = io["upt"].ap().rearrange("p (c i) -> p c i", c=CH)
    cmt_r = io["cmt"].ap().rearrange("p (c i) -> p c i", c=CH)
    uct_r = io["uct"].ap().rearrange("p (c k) -> p c k", c=CH)
    v4t_r = io["v4t"].ap().rearrange("p (c k) -> p c k", c=CH)
    wvr_r = io["wvr"].ap()

    import contextlib

    ctx = contextlib.ExitStack()
    singles = ctx.enter_context(tc.tile_pool(name="singles", bufs=1))
    # bufs=4 so the h4/h2 product tiles get distinct buffers -- with 2, the
    # h2 writes pick up WAR edges against the ncp matmuls' h4 reads, which
    # chains the zg path behind the nz path at the very end of the kernel.
    scratch = ctx.enter_context(tc.tile_pool(name="scratch", bufs=4))
    psum = ctx.enter_context(tc.tile_pool(name="psum", bufs=1, space="PSUM"))

    upt = singles.tile([128, CH, ISH], F8)
    asp = singles.tile([128, CH, ISH], F8)
    cmt = singles.tile([128, CH, ISH], F8)
    uct = singles.tile([128, CH, KSH], F8)
    v4t = singles.tile([128, CH, KSH], F8)
    wvr = singles.tile([128, CH * 2 + ITILES], F32)
    wv = wvr[:, : CH * 2].rearrange("p (c v) -> p c v", c=CH)
    rb = wvr[:, CH * 2 :]
    ones = singles.tile([128, 1], F16)

    ev = [singles.tile([128, KSH], F16, name=f"ev{it}", tag=f"ev{it}") for it in range(ITILES)]
    zg = singles.tile([128, 2 * ITILES], F32)  # cols: Z it0, Z it1, G it0, G it1
    nzs = singles.tile([1, 2 * KSH], F32)

    # PSUM: 6 full-kernel accumulators + 2 column-sum banks = all 8 banks.
    tps = [psum.tile([128, KSH], F32, name=f"tps{it}", tag=f"tps{it}") for it in range(ITILES)]
    mps = [psum.tile([128, KSH], F32, name=f"mps{it}", tag=f"mps{it}") for it in range(ITILES)]
    qps = [psum.tile([128, KSH], F32, name=f"qps{it}", tag=f"qps{it}") for it in range(ITILES)]
    zcp = psum.tile([1, KSH], F32)
    ncp = psum.tile([1, KSH], F32)

    # Warm-up operands first so the dummy matmuls can start right after the
    # prologue and have the HAM activity window at full clock by the time
    # real data lands.
    dum = singles.tile([128, KSH], F16)
    nc.gpsimd.memset(ones, 1.0)
    nc.gpsimd.memset(dum, 0.0)
    for _ in range(8):
        nc.tensor.matmul(zcp, lhsT=ones, rhs=dum, start=True, stop=True)

    # Input DMAs.  wvr leads on the sync HWDGE queue (it gates the asp
    # derivation; the gpsimd SWDGE queue's first byte lands ~4us later);
    # the bulk fp8 operands split ~1.5MB/1.5MB across the two HWDGE queues
    # in consumption order: upt/uct (S path) first, v4t next, cmt last.
    nc.sync.dma_start(out=wvr, in_=wvr_r)
    nc.scalar.dma_start(out=upt[:, 0:8, :], in_=upt_r[:, 0:8, :])
    for c0, c1 in ((0, 4), (4, 8), (8, 12), (12, 16)):
        nc.sync.dma_start(out=uct[:, c0:c1, :], in_=uct_r[:, c0:c1, :])
    nc.scalar.dma_start(out=upt[:, 8:16, :], in_=upt_r[:, 8:16, :])
    nc.scalar.dma_start(out=v4t[:, 0:4, :], in_=v4t_r[:, 0:4, :])
    nc.scalar.dma_start(out=v4t[:, 4:8, :], in_=v4t_r[:, 4:8, :])
    nc.scalar.dma_start(out=v4t[:, 8:12, :], in_=v4t_r[:, 8:12, :])
    nc.scalar.dma_start(out=v4t[:, 12:16, :], in_=v4t_r[:, 12:16, :])
    nc.sync.dma_start(out=cmt[:, 0:8, :], in_=cmt_r[:, 0:8, :])
    nc.sync.dma_start(out=cmt[:, 8:16, :], in_=cmt_r[:, 8:16, :])

    # asp = 16*(u_p*w3 + w2) per-partition scalars, chunk by chunk as upt
    # lands (DVE computes in f32 internally, converts fp8 in/out).
    for c in range(CH):
        nc.vector.tensor_scalar(
            out=asp[:, c, :],
            in0=upt[:, c, :],
            scalar1=wv[:, c, 0:1],
            scalar2=wv[:, c, 1:2],
            op0=mult,
            op1=add,
        )

    # S (tps) and M4 (qps) contractions interleaved per chunk pair: the T
    # matmuls pace on the serial asp chain, and the Q matmuls (which only
    # need upt/v4t, both early in the queues) fill the PE's asp-wait gaps.
    for c in range(0, CH, 2):
        st = c == 0
        sp = c == CH - 2
        for it in range(ITILES):
            lhs = asp[:, c : c + 2, it * 128 : (it + 1) * 128]
            nc.tensor.matmul(
                tps[it], lhsT=lhs, rhs=uct[:, c : c + 2, :],
                start=st, stop=sp, perf_mode=DR,
            )
        for it in range(ITILES):
            lhs = upt[:, c : c + 2, it * 128 : (it + 1) * 128]
            nc.tensor.matmul(
                qps[it], lhsT=lhs, rhs=v4t[:, c : c + 2, :],
                start=st, stop=sp, perf_mode=DR,
            )

    # E = exp(T/16 + r) as soon as tps closes; row sums via activation accum.
    for it in range(ITILES):
        nc.scalar.activation(
            out=ev[it],
            in_=tps[it],
            func=mybir.ActivationFunctionType.Exp,
            bias=rb[:, it : it + 1],
            scale=1.0 / SC_A,
            accum_out=zg[:, it : it + 1],
        )

    # Z' = colsum(E) and h4 = E * (512*M4) follow the exps; the M2 phase
    # below overlaps them on the PE.
    for jt in range(ITILES):
        nc.tensor.matmul(
            zcp, lhsT=ones, rhs=ev[jt],
            start=(jt == 0), stop=(jt == ITILES - 1),
        )
    nc.scalar.activation(
        out=nzs[:, 0:KSH], in_=zcp, func=mybir.ActivationFunctionType.Copy
    )
    h4s = []
    for it in range(ITILES):
        h4 = scratch.tile([128, KSH], F16, tag=f"h4_{it}")
        nc.vector.scalar_tensor_tensor(
            out=h4, in0=ev[it], scalar=1.0, in1=qps[it], op0=mult, op1=mult,
        )
        h4s.append(h4)

    # M2 contraction (mps), row-tile-separated so h2[0] overlaps M_it1;
    # N' = colsum(h4), its copy and the nz DMA close the PE/scalar streams
    # while only h2[1] + the zg DMA follow the final DR matmul.
    for it in range(ITILES):
        for c in range(0, CH, 2):
            lhs = cmt[:, c : c + 2, it * 128 : (it + 1) * 128]
            nc.tensor.matmul(
                mps[it], lhsT=lhs, rhs=uct[:, c : c + 2, :],
                start=(c == 0), stop=(c == CH - 2), perf_mode=DR,
            )
        h2 = scratch.tile([128, KSH], F16, tag=f"h2_{it}")
        nc.vector.scalar_tensor_tensor(
            out=h2,
            in0=ev[it],
            scalar=1.0,
            in1=mps[it],
            op0=mult,
            op1=mult,
            accum_out=zg[:, ITILES + it : ITILES + it + 1],
        )
    nc.sync.dma_start(out=io["out_zg"].ap(), in_=zg)
    for jt in range(ITILES):
        nc.tensor.matmul(
            ncp, lhsT=ones, rhs=h4s[jt],
            start=(jt == 0), stop=(jt == ITILES - 1),
        )
    nc.scalar.activation(
        out=nzs[:, KSH:], in_=ncp, func=mybir.ActivationFunctionType.Copy
    )
    nc.scalar.dma_start(out=io["out_nz"].ap(), in_=nzs)
    ctx.close()


def _build() -> bass.Bass:
    nc = bass.Bass()
    io = {}
    io["upt"] = nc.declare_dram_parameter("upt", [128, CH * ISH], F8, isOutput=False)
    io["cmt"] = nc.declare_dram_parameter("cmt", [128, CH * ISH], F8, isOutput=False)
    io["uct"] = nc.declare_dram_parameter("uct", [128, CH * KSH], F8, isOutput=False)
    io["v4t"] = nc.declare_dram_parameter("v4t", [128, CH * KSH], F8, isOutput=False)
    io["wvr"] = nc.declare_dram_parameter(
        "wvr", [128, CH * 2 + ITILES], F32, isOutput=False
    )
    io["out_zg"] = nc.declare_dram_parameter(
        "out_zg", [128, 2 * ITILES], F32, isOutput=True
    )
    io["out_nz"] = nc.declare_dram_parameter(
        "out_nz", [1, 2 * KSH], F32, isOutput=True
    )
    with tile.TileContext(nc) as tc:
        _emit(nc, tc, io)
    _legalize_waits(nc)
    return nc


_NC_CACHE: bass.Bass | None = None


def _get_nc() -> bass.Bass:
    global _NC_CACHE
    if _NC_CACHE is None:
        _NC_CACHE = _build()
    return _NC_CACHE


def _pack8(a2d: np.ndarray) -> np.ndarray:
    """[D, x] (d-major) f32 -> [128, ch, x] p-major fp8 so each partition's
    data is one contiguous DRAM run per chunk group."""
    d, x = a2d.shape
    ch = d // 128
    v = np.ascontiguousarray(a2d.reshape(ch, 128, x).transpose(1, 0, 2))
    return np.clip(v, -240.0, 240.0).astype(NPF8)


def _in_maps(u_p, u_c, w_a, ffn_w):
    W = ffn_w.reshape(N, 4, D)
    wa = w_a[0]
    w1, w2, w3 = wa[:D], wa[D : 2 * D], wa[2 * D :]

    C = SC_CV * (W[:, 1, :] + u_p * W[:, 2, :])  # [m, d] scaled
    V4 = SC_CV * (u_c * W[:, 3, :])  # [n, d] scaled
    r = u_p @ w1 - 6.0  # [m] row bias, shifted into fp16-exp range

    u_pT = _pack8(u_p.T)
    u_cT = _pack8(u_c.T)
    CT = _pack8(C.T)
    V4T = _pack8(V4.T)
    wv = np.ascontiguousarray(
        np.stack([SC_A * w3, SC_A * w2], axis=1)
        .reshape(CH, 128, 2)
        .transpose(1, 0, 2)
        .astype(np.float32)
    )

    maps = []
    for ii in range(NI):
        isl = slice(ISH * ii, ISH * (ii + 1))
        rbl = np.ascontiguousarray(
            r[isl].astype(np.float32).reshape(ITILES, 128).T
        )
        wvr = np.ascontiguousarray(
            np.concatenate([wv.reshape(128, CH * 2), rbl], axis=1)
        )
        upt = np.ascontiguousarray(u_pT[:, :, isl]).reshape(128, -1)
        cmt = np.ascontiguousarray(CT[:, :, isl]).reshape(128, -1)
        for kk in range(NK):
            ksl = slice(KSH * kk, KSH * (kk + 1))
            maps.append(
                {
                    "upt": upt,
                    "cmt": cmt,
                    "uct": np.ascontiguousarray(u_cT[:, :, ksl]).reshape(128, -1),
                    "v4t": np.ascontiguousarray(V4T[:, :, ksl]).reshape(128, -1),
                    "wvr": wvr,
                }
            )
    return maps


def _reduce(results: list[dict], term1: float, ffn_b) -> float:
    """Pre-relu scalar from the per-core partial sums, in float64."""
    total = term1
    # term23: per I shard, Z/G summed over the 2 K cores, then sum_i G/Z.
    for ii in range(NI):
        zg0 = results[ii * NK]["out_zg"].astype(np.float64)
        zg1 = results[ii * NK + 1]["out_zg"].astype(np.float64)
        z = zg0[:, :ITILES] + zg1[:, :ITILES]
        g = (zg0[:, ITILES:] + zg1[:, ITILES:]) / SC_CV
        total += (g / z).sum()
    # term4: per K shard, Z'/N summed over the 4 I cores, then sum_k N/Z'.
    for kk in range(NK):
        acc = np.zeros((2 * KSH,), np.float64)
        for ii in range(NI):
            acc += results[ii * NK + kk]["out_nz"][0].astype(np.float64)
        total += (acc[KSH:] / SC_CV / acc[:KSH]).sum()
    return total + float(np.asarray(ffn_b)[0])


def kernel(u_p, u_c, w_a, ffn_w, ffn_b, **run_kwargs):
    nc = _get_nc()
    u_p = np.asarray(u_p, np.float64)
    u_c = np.asarray(u_c, np.float64)
    w_a = np.asarray(w_a, np.float64)
    ffn_w = np.asarray(ffn_w, np.float64)
    maps = _in_maps(u_p, u_c, w_a, ffn_w)
    term1 = float(np.sum(u_p * ffn_w.reshape(N, 4, D)[:, 0, :]))
    res = run_bass_kernel_spmd(nc, maps, core_ids=list(range(NCORES)), **run_kwargs)
    out = np.array([max(_reduce(res.results, term1, ffn_b), 0.0)], dtype=np.float32)
    if run_kwargs:
        return out, res
    return out
